# revision 9
# baseline (speedup 1.0000x reference)
"""Trainium2 Bass kernel for nn_DingoNet_76854144795142.

v2 pipeline (per core, 16 of 128 batches, pure data parallel):
  host ships x3 = tap-shifted embedded sequence (bf16, 15 rows + ones row)
  -> conv as ONE K=16 bf16 matmul per (batch, section) with 4x4
     tile_position packing -> relu evac on DVE -> H bf16 (d-major,
     d' = s*32 + c)
  -> T = W_a' @ H (bf16, stationary weights) -> tanh (ACT) -> tt bf16
  -> e = v' @ tt (M=32 column-packed bf16 matmuls) -> exp folded into
     PSUM evac with accum_out => softmax sums (ACT)
  -> XBAR DMA-transposes (2-byte, SBUF->SBUF): Ht blocks [128n, 128d]
     from H, wT blocks [128n, 16] from the replicated exp(e) rows
  -> ctx^T accumulated on PE: lhsT = wT[:,0:1], rhs = Ht -> psum [1, 128]
     per batch, 16 accumulating matmuls
  -> normalization + inverse permutation on host.

Weight tables (gt2, permuted W_a / v) are precomputed on host in bf16.
"""

import os
import sys

import numpy as np

for _p in ("/opt/trn_rl_repo", "/root/.axon_site/_ro/trn_rl_repo"):
    if os.path.isdir(_p) and _p not in sys.path:
        sys.path.insert(0, _p)

# ---- problem constants (hardcoded per task contract) ----
B = 128          # total batch
L = 8194         # sequence length
PL = 8192        # conv output length
S = 4            # sections
NL = 2048        # positions per section
C = 32           # conv channels
V = 26           # vocab
D = 128          # C * S
NCORES = 8
BPC = B // NCORES      # 16 batches per core
NG = BPC // 4          # 4 groups of 4 batches
CHUNK = 512            # free-dim chunk
NCHUNK = NL // CHUNK   # 4 chunks per section
BLK = 128              # xbar/ctx block
NBLK = CHUNK // BLK    # 4 blocks per chunk
KR = 16                # conv contraction rows (15 taps + ones row)

_CACHE = {}


def _build_program():
    """Build + compile the Bass/Tile program once per process."""
    if "prog" in _CACHE:
        return _CACHE["prog"]

    from contextlib import ExitStack

    import concourse.bass as bass
    import concourse.tile as tile
    from concourse import bacc, mybir

    f32 = mybir.dt.float32
    bf16 = mybir.dt.bfloat16
    AF = mybir.ActivationFunctionType

    nc = bacc.Bacc(
        "TRN2",
        target_bir_lowering=False,
        debug=False,
        enable_asserts=True,
        num_devices=NCORES,
    )

    x3_d = nc.dram_tensor("x3", [BPC, KR, PL], bf16, kind="ExternalInput")
    gt2_d = nc.dram_tensor("gt2", [128, 32], bf16, kind="ExternalInput")
    wat_d = nc.dram_tensor("wat", [128, 128], bf16, kind="ExternalInput")
    vcol_d = nc.dram_tensor("vcol", [128, 32], bf16, kind="ExternalInput")
    ctxb_d = nc.dram_tensor("ctxb", [4, 512], f32, kind="ExternalOutput")
    zsum_d = nc.dram_tensor("zsum", [BPC, 1], f32, kind="ExternalOutput")

    with tile.TileContext(nc) as tc, ExitStack() as ctx:
        consts = ctx.enter_context(tc.tile_pool(name="consts", bufs=1))
        ohp = ctx.enter_context(tc.tile_pool(name="ohp", bufs=2))
        hpool = ctx.enter_context(tc.tile_pool(name="hpool", bufs=2))
        ttpool = ctx.enter_context(tc.tile_pool(name="ttpool", bufs=2))
        eepool = ctx.enter_context(tc.tile_pool(name="eepool", bufs=2))
        htpool = ctx.enter_context(tc.tile_pool(name="htpool", bufs=4))
        wtpool = ctx.enter_context(tc.tile_pool(name="wtpool", bufs=3))
        smallp = ctx.enter_context(tc.tile_pool(name="smallp", bufs=2))
        psum_y = ctx.enter_context(tc.tile_pool(name="psum_y", bufs=1, space="PSUM"))
        psum_t = ctx.enter_context(tc.tile_pool(name="psum_t", bufs=2, space="PSUM"))
        psum_e = ctx.enter_context(tc.tile_pool(name="psum_e", bufs=1, space="PSUM"))
        psum_c = ctx.enter_context(tc.tile_pool(name="psum_c", bufs=1, space="PSUM"))

        gt2_sb = consts.tile([128, 32], bf16)
        nc.sync.dma_start(gt2_sb[:, :], gt2_d.ap())
        wat_sb = consts.tile([128, 128], bf16)
        nc.sync.dma_start(wat_sb[:, :], wat_d.ap())
        vcol_sb = consts.tile([128, 32], bf16)
        nc.sync.dma_start(vcol_sb[:, :], vcol_d.ap())

        x3_ap = x3_d.ap()

        def strided4(t, fr=None):
            # view of tile t at partitions {0,32,64,96}
            a = t[:, :] if fr is None else t[:, fr]
            pitch = a.ap[0][0]
            return bass.AP(
                tensor=a.tensor,
                offset=a.offset,
                ap=[[pitch * 32, 4]] + list(a.ap[1:]),
            )

        # ctx^T accumulator: batch (g, i) -> partition 32*i, cols 128*g..128*(g+1)
        ctxp = psum_c.tile([128, 512], f32, tag="ctxp", name="ctxp")

        def emit_x3_dma(g, oh):
            # one DMA per (batch, section): [16 rows, 2048 cols]
            for i in range(4):
                for j in range(S):
                    src = bass.AP(
                        tensor=x3_ap.tensor,
                        offset=(4 * g + i) * KR * PL + NL * j,
                        ap=[[PL, KR], [1, NL]],
                    )
                    nc.sync.dma_start(
                        out=oh[32 * i : 32 * i + KR, j, :], in_=src
                    )

        ohs = {}
        hps = {}
        tts = {}
        ees = {}
        zparts = {}
        hts = {}
        wts = {}

        def unit_conv(u):
            g, q = divmod(u, NCHUNK)
            if q == 0:
                ohs[g] = ohp.tile([128, S, NL], bf16, tag="oh", name="oh")
                emit_x3_dma(g, ohs[g])
                hps[g] = [
                    hpool.tile([128, NL], bf16, tag=f"hp{_i}", name=f"hp{_i}")
                    for _i in range(4)
                ]
                zparts[g] = smallp.tile([128, NCHUNK], f32, tag="zpart", name="zpart")
            oh = ohs[g]
            yps = [
                psum_y.tile([128, CHUNK], f32, tag=f"yp{_i}", name=f"yp{_i}")
                for _i in range(4)
            ]
            for j in range(S):
                for i in range(4):
                    nc.tensor.matmul(
                        out=yps[i][32 * j : 32 * j + 32, :],
                        lhsT=gt2_sb[32 * i : 32 * i + KR, :],
                        rhs=oh[32 * i : 32 * i + KR, j, CHUNK * q : CHUNK * (q + 1)],
                        start=True,
                        stop=True,
                        skip_group_check=True,
                        tile_position=(32 * i, 32 * j),
                    )
            for i in range(4):
                dst = hps[g][i][:, CHUNK * q : CHUNK * (q + 1)]
                nc.vector.tensor_scalar_max(dst, yps[i][:, :], 0.0)
            # H^T blocks for ctx, via XBAR transpose (reads DVE relu output)
            hts[u] = {}
            for i in range(4):
                for b in range(NBLK):
                    ht = htpool.tile(
                        [BLK, BLK], bf16, tag=f"ht{i}_{b}", name=f"ht{i}_{b}"
                    )
                    nc.sync.dma_start_transpose(
                        out=ht[:, :],
                        in_=hps[g][i][:, CHUNK * q + BLK * b : CHUNK * q + BLK * (b + 1)],
                    )
                    hts[u][(i, b)] = ht

        def unit_T(u):
            g, q = divmod(u, NCHUNK)
            hp = hps[g]
            tts[u] = []
            for i in range(4):
                tp = psum_t.tile([128, CHUNK], f32, tag="tp", name="tp")
                nc.tensor.matmul(
                    out=tp[:, :],
                    lhsT=wat_sb[:, :],
                    rhs=hp[i][:, CHUNK * q : CHUNK * (q + 1)],
                    start=True,
                    stop=True,
                )
                ttc = ttpool.tile([128, CHUNK], bf16, tag=f"tt{i}", name=f"tt{i}")
                nc.scalar.activation(ttc[:, :], tp[:, :], AF.Tanh)
                tts[u].append(ttc)

        def unit_e(u):
            g, q = divmod(u, NCHUNK)
            zpart = zparts[g]
            ttq = tts.pop(u)
            ep = psum_e.tile([128, CHUNK], f32, tag="ep", name="ep")
            for j in range(4):
                nc.tensor.matmul(
                    out=ep[32 * j : 32 * j + 32, :],
                    lhsT=vcol_sb[:, :],
                    rhs=ttq[j][:, :],
                    start=True,
                    stop=True,
                    tile_position=(0, 32 * j),
                )
            ee = eepool.tile([128, CHUNK], bf16, tag="ee", name="ee")
            nc.scalar.activation(
                ee[:, :], ep[:, :], AF.Exp, accum_out=zpart[:, q : q + 1]
            )
            ees[u] = ee
            # w^T blocks [128 n, 16 junk-replicated cols] via XBAR transpose
            wts[u] = {}
            for i in range(4):
                for b in range(NBLK):
                    wt = wtpool.tile(
                        [BLK, 16], bf16, tag=f"wt{i}_{b}", name=f"wt{i}_{b}"
                    )
                    nc.sync.dma_start_transpose(
                        out=wt[:, :],
                        in_=ee[32 * i : 32 * i + 16, BLK * b : BLK * (b + 1)],
                    )
                    wts[u][(i, b)] = wt
            if q == NCHUNK - 1:
                # group tail: zsum for the 4 batches
                zsum_sb = smallp.tile([128, 1], f32, tag="zsum", name="zsum")
                zscr = smallp.tile([128, NCHUNK], f32, tag="zscr", name="zscr")
                nc.scalar.activation(
                    zscr[:, :], zpart[:, :], AF.Copy, accum_out=zsum_sb[:, :]
                )
                nc.scalar.dma_start(
                    out=zsum_d.ap()[4 * g : 4 * g + 4, :], in_=strided4(zsum_sb)
                )

        def unit_ctx(u):
            g, q = divmod(u, NCHUNK)
            ht_u = hts.pop(u)
            wt_u = wts.pop(u)
            ees.pop(u, None)
            for i in range(4):
                for b in range(NBLK):
                    nc.tensor.matmul(
                        out=ctxp[32 * i : 32 * i + 1, 128 * g : 128 * (g + 1)],
                        lhsT=wt_u[(i, b)][:, 0:1],
                        rhs=ht_u[(i, b)][:, :],
                        start=(q == 0 and b == 0),
                        stop=(q == NCHUNK - 1 and b == NBLK - 1),
                        skip_group_check=True,
                        tile_position=(0, 32 * i),
                    )

        NU = NG * NCHUNK  # 16 units
        for u in range(NU + 3):
            if 1 <= u < NU + 1:
                unit_T(u - 1)
            if 2 <= u < NU + 2:
                unit_e(u - 2)
            if 3 <= u < NU + 3:
                unit_ctx(u - 3)
            if u < NU:
                unit_conv(u)

        ctx_sb = smallp.tile([128, 512], f32, tag="ctxsb", name="ctxsb")
        nc.scalar.activation(ctx_sb[:, :], ctxp[:, :], AF.Copy)
        nc.sync.dma_start(out=ctxb_d.ap(), in_=strided4(ctx_sb))

    nc.compile()
    _CACHE["prog"] = nc
    return nc


def _host_consts(emb, conv_w, conv_b, W_a, v_a):
    import ml_dtypes

    # permutation: d' = s*32 + c  <->  d = c*4 + s
    perm = np.array([(dp % 32) * 4 + dp // 32 for dp in range(128)], dtype=np.int64)
    # gt2 rows r = 3*i + k hold conv_w[c, i, k]; row 15 = conv_b
    blk = np.zeros((32, 32), dtype=np.float32)
    blk[:15, :] = conv_w.astype(np.float32).transpose(1, 2, 0).reshape(15, 32)
    blk[15, :] = conv_b.astype(np.float32)
    gt2 = np.tile(blk, (4, 1)).astype(ml_dtypes.bfloat16)  # [128, 32]
    W_p = W_a[np.ix_(perm, perm)].astype(np.float32)
    wat = np.ascontiguousarray(W_p.T).astype(ml_dtypes.bfloat16)
    vcol = np.ascontiguousarray(
        np.repeat(v_a[perm].astype(np.float32)[:, None], 32, axis=1)
    ).astype(ml_dtypes.bfloat16)
    return perm, gt2, wat, vcol


def _in_maps(input_seq, emb, conv_w, conv_b, W_a, v_a):
    import ml_dtypes

    perm, gt2, wat, vcol = _host_consts(emb, conv_w, conv_b, W_a, v_a)
    seq = np.asarray(input_seq).astype(np.int64)
    embf = np.asarray(emb).astype(np.float32)
    # x[b, i, p] = emb[seq[b, p], i]
    x = embf[seq]  # [B, L, 5]
    x3 = np.empty((B, KR, PL), dtype=ml_dtypes.bfloat16)
    for i in range(5):
        for k in range(3):
            x3[:, 3 * i + k, :] = x[:, k : k + PL, i].astype(ml_dtypes.bfloat16)
    x3[:, 15, :] = np.float32(1.0)
    maps = []
    for c in range(NCORES):
        maps.append(
            {
                "x3": np.ascontiguousarray(x3[BPC * c : BPC * (c + 1)]),
                "gt2": gt2,
                "wat": wat,
                "vcol": vcol,
            }
        )
    return perm, maps


def _assemble_one(perm, res):
    """ctx for one core: ctxb [4, 512] = [i, 128*g + d'] in d'-space."""
    buf = np.asarray(res["ctxb"], dtype=np.float32)
    ctxb = buf.reshape(4, 4, 128).transpose(1, 0, 2).reshape(BPC, D)
    zsum = np.asarray(res["zsum"], dtype=np.float32)[:, 0]
    blk = np.empty((BPC, D), dtype=np.float32)
    blk[:, perm] = ctxb / zsum[:, None]
    return blk


def _assemble(perm, results):
    out = np.empty((B, D), dtype=np.float32)
    for c, res in enumerate(results):
        out[BPC * c : BPC * (c + 1)] = _assemble_one(perm, res)
    return out


def kernel(input_seq, emb, conv_w, conv_b, W_a, v_a):
    from concourse import bass_utils

    nc = _build_program()
    perm, maps = _in_maps(input_seq, emb, conv_w, conv_b, W_a, v_a)
    res = bass_utils.run_bass_kernel_spmd(nc, maps, core_ids=list(range(NCORES)))
    return _assemble(perm, res.results)


# revision 14
# speedup vs baseline: 6.0718x; 6.0718x over previous
"""Trainium2 Bass kernel for nn_DingoNet_76854144795142.

v3 pipeline (per core, 16 of 128 batches, pure data parallel):
  host ships x3 = tap-shifted embedded sequence (bf16, 15 rows + ones row)
  -> conv as ONE K=64 bf16 matmul per (batch, chunk): block-diagonal
     weights produce all 4 sections (full M=128 = d' = s*32 + c) at once
  -> relu evac on DVE -> H bf16 [128 d', 4 batch, 2048 n]
  -> T = W_a' @ H (bf16, stationary weights) -> tanh (ACT) -> tt bf16
  -> e^T on PE: lhsT = tt n-block [128 d, 128 n], rhs = v [128 d, 1]
     -> etp psum [128 n, 16 cols] -> exp evac (ACT, tiny) -> wt_all bf16
  -> H^T via one XBAR DMA-transpose per (group, batch):
     [128 d, 2048 n] -> Ht [128 n, 16 blk, 128 d]
  -> ctx^T on PE: lhsT = wt col [128 n, 1], rhs = Ht blk [128 n, 128 d]
     -> psum row per batch, 16 accumulating matmuls
  -> softmax normalization on host (zsum = column sums of wt dump).
"""

import os
import sys

import numpy as np

for _p in ("/opt/trn_rl_repo", "/root/.axon_site/_ro/trn_rl_repo"):
    if os.path.isdir(_p) and _p not in sys.path:
        sys.path.insert(0, _p)

# ---- problem constants (hardcoded per task contract) ----
B = 128          # total batch
L = 8194         # sequence length
PL = 8192        # conv output length
S = 4            # sections
NL = 2048        # positions per section
C = 32           # conv channels
V = 26           # vocab
D = 128          # C * S
NCORES = 8
BPC = B // NCORES      # 16 batches per core
NG = BPC // 4          # 4 groups of 4 batches
CHUNK = 512            # free-dim chunk
NCHUNK = NL // CHUNK   # 4 chunks per section
BLK = 128              # xbar/ctx block
NBLK = CHUNK // BLK    # 4 blocks per chunk
KR = 16                # conv taps per section (15 + ones row)
NU = NG * NCHUNK       # 16 units
CTX_LAG = 5            # units between conv(u) and ctx(u)

_CACHE = {}


def _build_program():
    """Build + compile the Bass/Tile program once per process."""
    if "prog" in _CACHE:
        return _CACHE["prog"]

    from contextlib import ExitStack

    import concourse.bass as bass
    import concourse.tile as tile
    from concourse import bacc, mybir

    f32 = mybir.dt.float32
    bf16 = mybir.dt.bfloat16
    AF = mybir.ActivationFunctionType

    nc = bacc.Bacc(
        "TRN2",
        target_bir_lowering=False,
        debug=False,
        enable_asserts=True,
        num_devices=NCORES,
    )

    x3_d = nc.dram_tensor("x3", [BPC, KR, PL], bf16, kind="ExternalInput")
    gt3_d = nc.dram_tensor("gt3", [128, 128], bf16, kind="ExternalInput")
    wat_d = nc.dram_tensor("wat", [128, 128], bf16, kind="ExternalInput")
    vone_d = nc.dram_tensor("vone", [128, 1], bf16, kind="ExternalInput")
    ctxb_d = nc.dram_tensor("ctxb", [4, 512], f32, kind="ExternalOutput")
    wdump_d = nc.dram_tensor("wdump", [128, 16 * NU], bf16, kind="ExternalOutput")

    with tile.TileContext(nc) as tc, ExitStack() as ctx:
        consts = ctx.enter_context(tc.tile_pool(name="consts", bufs=1))
        ohp = ctx.enter_context(tc.tile_pool(name="ohp", bufs=2))
        hpool = ctx.enter_context(tc.tile_pool(name="hpool", bufs=2))
        ttpool = ctx.enter_context(tc.tile_pool(name="ttpool", bufs=2))
        htpool = ctx.enter_context(tc.tile_pool(name="htpool", bufs=2))
        smallp = ctx.enter_context(tc.tile_pool(name="smallp", bufs=2))
        wallp = ctx.enter_context(tc.tile_pool(name="wallp", bufs=1))
        psum_y = ctx.enter_context(tc.tile_pool(name="psum_y", bufs=1, space="PSUM"))
        psum_t = ctx.enter_context(tc.tile_pool(name="psum_t", bufs=2, space="PSUM"))
        psum_e = ctx.enter_context(tc.tile_pool(name="psum_e", bufs=1, space="PSUM"))
        psum_c = ctx.enter_context(tc.tile_pool(name="psum_c", bufs=1, space="PSUM"))

        gt3_sb = consts.tile([128, 128], bf16)
        nc.sync.dma_start(gt3_sb[:, :], gt3_d.ap())
        wat_sb = consts.tile([128, 128], bf16)
        nc.sync.dma_start(wat_sb[:, :], wat_d.ap())
        vone_sb = consts.tile([128, 1], bf16)
        nc.sync.dma_start(vone_sb[:, :], vone_d.ap())

        x3_ap = x3_d.ap()

        def strided4(t, fr=None):
            a = t[:, :] if fr is None else t[:, fr]
            pitch = a.ap[0][0]
            return bass.AP(
                tensor=a.tensor,
                offset=a.offset,
                ap=[[pitch * 32, 4]] + list(a.ap[1:]),
            )

        # ctx^T accumulator: batch (g, i) -> partition 32*i, cols 128*g..
        ctxp = psum_c.tile([128, 512], f32, tag="ctxp", name="ctxp")
        # exp(e^T) for all units: cols 16*u + 4*i + b
        wt_all = wallp.tile([128, 16 * NU], bf16, tag="wtall", name="wtall")

        def emit_x3_dma(g, ohts):
            # one DMA per batch: 64 rows = (section j, tap t), 2048 cols
            for i in range(4):
                src = bass.AP(
                    tensor=x3_ap.tensor,
                    offset=(4 * g + i) * KR * PL,
                    ap=[[NL, S], [PL, KR], [1, NL]],
                )
                dst = ohts[i // 2][64 * (i % 2) : 64 * (i % 2) + 64, :]
                nc.sync.dma_start(out=dst, in_=src)

        ohs = {}
        hps = {}
        tts = {}
        hts = {}

        def unit_conv(u):
            g, q = divmod(u, NCHUNK)
            if q == 0:
                ohs[g] = [
                    ohp.tile([128, NL], bf16, tag=f"oh{_t}", name=f"oh{_t}")
                    for _t in range(2)
                ]
                emit_x3_dma(g, ohs[g])
                hps[g] = hpool.tile([128, 4, NL], bf16, tag="hp", name="hp")
            yps = [
                psum_y.tile([128, CHUNK], f32, tag=f"yp{_i}", name=f"yp{_i}")
                for _i in range(4)
            ]
            for i in range(4):
                base = 64 * (i % 2)
                nc.tensor.matmul(
                    out=yps[i][:, :],
                    lhsT=gt3_sb[base : base + 64, :],
                    rhs=ohs[g][i // 2][
                        base : base + 64, CHUNK * q : CHUNK * (q + 1)
                    ],
                    start=True,
                    stop=True,
                    skip_group_check=True,
                    tile_position=(base, 0),
                )
            for i in range(4):
                nc.vector.tensor_scalar_max(
                    hps[g][:, i, CHUNK * q : CHUNK * (q + 1)], yps[i][:, :], 0.0
                )
            if q == NCHUNK - 1:
                # whole-group H ready: one XBAR transpose per batch
                hts[g] = []
                for i in range(4):
                    ht = htpool.tile(
                        [BLK, 16, BLK], bf16, tag=f"ht{i}", name=f"ht{i}"
                    )
                    nc.sync.dma_start_transpose(
                        out=ht[:, :, :], in_=hps[g][:, i, :]
                    )
                    hts[g].append(ht)

        def unit_T(u):
            g, q = divmod(u, NCHUNK)
            hp = hps[g]
            tts[u] = []
            for i in range(4):
                tp = psum_t.tile([128, CHUNK], f32, tag="tp", name="tp")
                nc.tensor.matmul(
                    out=tp[:, :],
                    lhsT=wat_sb[:, :],
                    rhs=hp[:, i, CHUNK * q : CHUNK * (q + 1)],
                    start=True,
                    stop=True,
                )
                ttc = ttpool.tile([128, CHUNK], bf16, tag=f"tt{i}", name=f"tt{i}")
                nc.scalar.activation(ttc[:, :], tp[:, :], AF.Tanh)
                tts[u].append(ttc)

        def unit_eT(u):
            ttq = tts.pop(u)
            etp = psum_e.tile([128, 16], f32, tag="etp", name="etp")
            for i in range(4):
                for b in range(NBLK):
                    col = 4 * i + b
                    nc.tensor.matmul(
                        out=etp[:, col : col + 1],
                        lhsT=ttq[i][:, BLK * b : BLK * (b + 1)],
                        rhs=vone_sb[:, :],
                        start=True,
                        stop=True,
                        skip_group_check=True,
                    )
            nc.scalar.activation(
                wt_all[:, 16 * u : 16 * (u + 1)], etp[:, :], AF.Exp
            )

        def unit_ctx(u):
            g, q = divmod(u, NCHUNK)
            ht_g = hts[g]
            for i in range(4):
                for b in range(NBLK):
                    col = 16 * u + 4 * i + b
                    nb = NBLK * q + b  # global 128-block within section run
                    nc.tensor.matmul(
                        out=ctxp[32 * i : 32 * i + 1, 128 * g : 128 * (g + 1)],
                        lhsT=wt_all[:, col : col + 1],
                        rhs=ht_g[i][:, nb, :],
                        start=(q == 0 and b == 0),
                        stop=(q == NCHUNK - 1 and b == NBLK - 1),
                        skip_group_check=True,
                        tile_position=(0, 32 * i),
                    )
            if q == NCHUNK - 1:
                hts.pop(g)

        for u in range(NU + CTX_LAG):
            if 1 <= u < NU + 1:
                unit_T(u - 1)
            if 2 <= u < NU + 2:
                unit_eT(u - 2)
            if u >= CTX_LAG and u - CTX_LAG < NU:
                unit_ctx(u - CTX_LAG)
            if u < NU:
                unit_conv(u)

        ctx_sb = smallp.tile([128, 512], f32, tag="ctxsb", name="ctxsb")
        for i in range(4):
            nc.scalar.activation(
                ctx_sb[32 * i : 32 * i + 1, :], ctxp[32 * i : 32 * i + 1, :], AF.Copy
            )
        nc.sync.dma_start(out=ctxb_d.ap(), in_=strided4(ctx_sb))
        nc.sync.dma_start(out=wdump_d.ap(), in_=wt_all[:, :])

    nc.compile()
    _CACHE["prog"] = nc
    return nc


def _host_consts(emb, conv_w, conv_b, W_a, v_a):
    import ml_dtypes

    # permutation: d' = s*32 + c  <->  d = c*4 + s
    perm = np.array([(dp % 32) * 4 + dp // 32 for dp in range(128)], dtype=np.int64)
    # block-diagonal conv weights: rows (j, 3i+k) -> cols (j, c)
    w15 = conv_w.astype(np.float32).transpose(1, 2, 0).reshape(15, 32)
    W3 = np.zeros((64, 128), dtype=np.float32)
    for j in range(S):
        W3[16 * j : 16 * j + 15, 32 * j : 32 * j + 32] = w15
        W3[16 * j + 15, 32 * j : 32 * j + 32] = conv_b.astype(np.float32)
    gt3 = np.vstack([W3, W3]).astype(ml_dtypes.bfloat16)  # [128, 128]
    W_p = W_a[np.ix_(perm, perm)].astype(np.float32)
    wat = np.ascontiguousarray(W_p.T).astype(ml_dtypes.bfloat16)
    vone = np.ascontiguousarray(
        v_a[perm].astype(np.float32)[:, None]
    ).astype(ml_dtypes.bfloat16)
    return perm, gt3, wat, vone


def _in_maps(input_seq, emb, conv_w, conv_b, W_a, v_a):
    import ml_dtypes

    perm, gt3, wat, vone = _host_consts(emb, conv_w, conv_b, W_a, v_a)
    seq = np.asarray(input_seq).astype(np.int64)
    embf = np.asarray(emb).astype(np.float32)
    x = embf[seq]  # [B, L, 5]
    x3 = np.empty((B, KR, PL), dtype=ml_dtypes.bfloat16)
    for i in range(5):
        for k in range(3):
            x3[:, 3 * i + k, :] = x[:, k : k + PL, i].astype(ml_dtypes.bfloat16)
    x3[:, 15, :] = np.float32(1.0)
    maps = []
    for c in range(NCORES):
        maps.append(
            {
                "x3": np.ascontiguousarray(x3[BPC * c : BPC * (c + 1)]),
                "gt3": gt3,
                "wat": wat,
                "vone": vone,
            }
        )
    return perm, maps


def _assemble_one(perm, res):
    """ctx for one core: ctxb [4, 512] = [i, 128*g + d'], wdump [128, 16*NU]."""
    buf = np.asarray(res["ctxb"], dtype=np.float32)
    ctxb = buf.reshape(4, 4, 128).transpose(1, 0, 2).reshape(BPC, D)
    wd = np.asarray(res["wdump"], dtype=np.float32)
    # col 16*(4g+q) + 4i + b -> batch 4g+i
    wr = wd.reshape(128, NG, NCHUNK, 4, NBLK)  # [n', g, q, i, b]
    zsum = wr.sum(axis=(0, 2, 4)).reshape(BPC)  # [g, i] row-major -> batch 4g+i
    blk = np.empty((BPC, D), dtype=np.float32)
    blk[:, perm] = ctxb / zsum[:, None]
    return blk


def _assemble(perm, results):
    out = np.empty((B, D), dtype=np.float32)
    for c, res in enumerate(results):
        out[BPC * c : BPC * (c + 1)] = _assemble_one(perm, res)
    return out


def kernel(input_seq, emb, conv_w, conv_b, W_a, v_a):
    from concourse import bass_utils

    nc = _build_program()
    perm, maps = _in_maps(input_seq, emb, conv_w, conv_b, W_a, v_a)
    res = bass_utils.run_bass_kernel_spmd(nc, maps, core_ids=list(range(NCORES)))
    return _assemble(perm, res.results)


# revision 22
# speedup vs baseline: 6.2110x; 1.0229x over previous
"""Trainium2 Bass kernel for nn_DingoNet_76854144795142.

v3 pipeline (per core, 16 of 128 batches, pure data parallel):
  host ships x3 = tap-shifted embedded sequence (bf16, 15 rows + ones row)
  -> conv as ONE K=64 bf16 matmul per (batch, chunk): block-diagonal
     weights produce all 4 sections (full M=128 = d' = s*32 + c) at once
  -> relu evac on DVE -> H bf16 [128 d', 4 batch, 2048 n]
  -> T = W_a' @ H (bf16, stationary weights) -> tanh (ACT) -> tt bf16
  -> e^T on PE: lhsT = tt n-block [128 d, 128 n], rhs = v [128 d, 1]
     -> etp psum [128 n, 16 cols] -> exp evac (ACT, tiny) -> wt_all bf16
  -> H^T via one XBAR DMA-transpose per (group, batch):
     [128 d, 2048 n] -> Ht [128 n, 16 blk, 128 d]
  -> ctx^T on PE: lhsT = wt col [128 n, 1], rhs = Ht blk [128 n, 128 d]
     -> psum row per batch, 16 accumulating matmuls
  -> softmax normalization on host (zsum = column sums of wt dump).
"""

import os
import sys

import numpy as np

for _p in ("/opt/trn_rl_repo", "/root/.axon_site/_ro/trn_rl_repo"):
    if os.path.isdir(_p) and _p not in sys.path:
        sys.path.insert(0, _p)

# ---- problem constants (hardcoded per task contract) ----
B = 128          # total batch
L = 8194         # sequence length
PL = 8192        # conv output length
S = 4            # sections
NL = 2048        # positions per section
C = 32           # conv channels
V = 26           # vocab
D = 128          # C * S
NCORES = 8
BPC = B // NCORES      # 16 batches per core
NG = BPC // 4          # 4 groups of 4 batches
CHUNK = 512            # free-dim chunk
NCHUNK = NL // CHUNK   # 4 chunks per section
BLK = 128              # xbar/ctx block
NBLK = CHUNK // BLK    # 4 blocks per chunk
KR = 16                # conv taps per section (15 + ones row)
NU = NG * NCHUNK       # 16 units
CTX_LAG = 6            # units between conv(u) and ctx(u)

_CACHE = {}


def _build_program():
    """Build + compile the Bass/Tile program once per process."""
    if "prog" in _CACHE:
        return _CACHE["prog"]

    from contextlib import ExitStack

    import concourse.bass as bass
    import concourse.tile as tile
    from concourse import bacc, mybir

    f32 = mybir.dt.float32
    bf16 = mybir.dt.bfloat16
    AF = mybir.ActivationFunctionType

    nc = bacc.Bacc(
        "TRN2",
        target_bir_lowering=False,
        debug=False,
        enable_asserts=True,
        num_devices=NCORES,
    )

    x3_d = nc.dram_tensor("x3", [BPC, KR, PL], bf16, kind="ExternalInput")
    gt3_d = nc.dram_tensor("gt3", [128, 128], bf16, kind="ExternalInput")
    wat_d = nc.dram_tensor("wat", [128, 128], bf16, kind="ExternalInput")
    vone_d = nc.dram_tensor("vone", [128, 1], bf16, kind="ExternalInput")
    ctxb_d = nc.dram_tensor("ctxb", [4, 512], f32, kind="ExternalOutput")
    wdump_d = nc.dram_tensor("wdump", [128, 16 * NU], bf16, kind="ExternalOutput")

    with tile.TileContext(nc) as tc, ExitStack() as ctx:
        consts = ctx.enter_context(tc.tile_pool(name="consts", bufs=1))
        ohp = ctx.enter_context(tc.tile_pool(name="ohp", bufs=2))
        hpool = ctx.enter_context(tc.tile_pool(name="hpool", bufs=2))
        ttpool = ctx.enter_context(tc.tile_pool(name="ttpool", bufs=2))
        htpool = ctx.enter_context(tc.tile_pool(name="htpool", bufs=2))
        smallp = ctx.enter_context(tc.tile_pool(name="smallp", bufs=2))
        wallp = ctx.enter_context(tc.tile_pool(name="wallp", bufs=1))
        psum_y = ctx.enter_context(tc.tile_pool(name="psum_y", bufs=1, space="PSUM"))
        psum_t = ctx.enter_context(tc.tile_pool(name="psum_t", bufs=2, space="PSUM"))
        psum_e = ctx.enter_context(tc.tile_pool(name="psum_e", bufs=1, space="PSUM"))
        psum_c = ctx.enter_context(tc.tile_pool(name="psum_c", bufs=1, space="PSUM"))

        gt3_sb = consts.tile([128, 128], bf16)
        nc.sync.dma_start(gt3_sb[:, :], gt3_d.ap())
        wat_sb = consts.tile([128, 128], bf16)
        nc.sync.dma_start(wat_sb[:, :], wat_d.ap())
        vone_sb = consts.tile([128, 1], bf16)
        nc.sync.dma_start(vone_sb[:, :], vone_d.ap())

        x3_ap = x3_d.ap()

        def strided4(t, fr=None):
            a = t[:, :] if fr is None else t[:, fr]
            pitch = a.ap[0][0]
            return bass.AP(
                tensor=a.tensor,
                offset=a.offset,
                ap=[[pitch * 32, 4]] + list(a.ap[1:]),
            )

        # ctx^T accumulator: batch (g, i) -> partition 32*i, cols 128*g..
        ctxp = psum_c.tile([128, 512], f32, tag="ctxp", name="ctxp")
        # exp(e^T) for all units: cols 16*u + 4*i + b
        wt_all = wallp.tile([128, 16 * NU], bf16, tag="wtall", name="wtall")

        def emit_x3_dma(g, ohts, split_first):
            # per batch: 64 rows = (section j, tap t); optionally split off
            # the chunk-0 columns so the first conv can start early
            for i in range(4):
                base = (4 * g + i) * KR * PL
                lo = 64 * (i % 2)
                dst = ohts[i // 2]
                if split_first:
                    src0 = bass.AP(
                        tensor=x3_ap.tensor,
                        offset=base,
                        ap=[[NL, S], [PL, KR], [1, CHUNK]],
                    )
                    nc.sync.dma_start(
                        out=dst[lo : lo + 64, 0:CHUNK], in_=src0
                    )
                    src1 = bass.AP(
                        tensor=x3_ap.tensor,
                        offset=base + CHUNK,
                        ap=[[NL, S], [PL, KR], [1, NL - CHUNK]],
                    )
                    nc.sync.dma_start(
                        out=dst[lo : lo + 64, CHUNK:NL], in_=src1
                    )
                else:
                    src = bass.AP(
                        tensor=x3_ap.tensor,
                        offset=base,
                        ap=[[NL, S], [PL, KR], [1, NL]],
                    )
                    nc.sync.dma_start(out=dst[lo : lo + 64, :], in_=src)

        ohs = {}
        hps = {}
        tts = {}
        hts = {}

        def prefetch_x3(g):
            ohs[g] = [
                ohp.tile([128, NL], bf16, tag=f"oh{_t}", name=f"oh{_t}")
                for _t in range(2)
            ]
            emit_x3_dma(g, ohs[g], split_first=(g == 0))

        def unit_conv(u):
            g, q = divmod(u, NCHUNK)
            if q == 1 and g + 1 < NG:
                prefetch_x3(g + 1)
            if q == 0:
                hps[g] = hpool.tile([128, 4, NL], bf16, tag="hp", name="hp")
            yps = [
                psum_y.tile([128, CHUNK], f32, tag=f"yp{_i}", name=f"yp{_i}")
                for _i in range(4)
            ]
            for i in range(4):
                base = 64 * (i % 2)
                nc.tensor.matmul(
                    out=yps[i][:, :],
                    lhsT=gt3_sb[base : base + 64, :],
                    rhs=ohs[g][i // 2][
                        base : base + 64, CHUNK * q : CHUNK * (q + 1)
                    ],
                    start=True,
                    stop=True,
                    skip_group_check=True,
                    tile_position=(base, 0),
                )
            for i in range(4):
                nc.vector.tensor_scalar_max(
                    hps[g][:, i, CHUNK * q : CHUNK * (q + 1)], yps[i][:, :], 0.0
                )
            if q == NCHUNK - 1:
                # whole-group H ready: one XBAR transpose per batch
                hts[g] = []
                for i in range(4):
                    ht = htpool.tile(
                        [BLK, 16, BLK], bf16, tag=f"ht{i}", name=f"ht{i}"
                    )
                    nc.sync.dma_start_transpose(
                        out=ht[:, :, :], in_=hps[g][:, i, :]
                    )
                    hts[g].append(ht)

        def unit_T(u):
            g, q = divmod(u, NCHUNK)
            hp = hps[g]
            tts[u] = []
            for i in range(4):
                tp = psum_t.tile([128, CHUNK], f32, tag="tp", name="tp")
                nc.tensor.matmul(
                    out=tp[:, :],
                    lhsT=wat_sb[:, :],
                    rhs=hp[:, i, CHUNK * q : CHUNK * (q + 1)],
                    start=True,
                    stop=True,
                )
                ttc = ttpool.tile([128, CHUNK], bf16, tag=f"tt{i}", name=f"tt{i}")
                nc.scalar.activation(ttc[:, :], tp[:, :], AF.Tanh)
                tts[u].append(ttc)

        def unit_eT(u):
            ttq = tts.pop(u)
            etp = psum_e.tile([128, 16], f32, tag="etp", name="etp")
            for i in range(4):
                for b in range(NBLK):
                    col = 4 * i + b
                    nc.tensor.matmul(
                        out=etp[:, col : col + 1],
                        lhsT=ttq[i][:, BLK * b : BLK * (b + 1)],
                        rhs=vone_sb[:, :],
                        start=True,
                        stop=True,
                        skip_group_check=True,
                    )
            nc.scalar.activation(
                wt_all[:, 16 * u : 16 * (u + 1)], etp[:, :], AF.Exp
            )
            g, q = divmod(u, NCHUNK)
            if q == NCHUNK - 1:
                # dump this group's softmax numerators early (host sums Z)
                nc.scalar.dma_start(
                    out=wdump_d.ap()[:, 64 * g : 64 * (g + 1)],
                    in_=wt_all[:, 64 * g : 64 * (g + 1)],
                )

        def unit_ctx(u):
            g, q = divmod(u, NCHUNK)
            ht_g = hts[g]
            for i in range(4):
                for b in range(NBLK):
                    col = 16 * u + 4 * i + b
                    nb = NBLK * q + b  # global 128-block within section run
                    nc.tensor.matmul(
                        out=ctxp[32 * i : 32 * i + 1, 128 * g : 128 * (g + 1)],
                        lhsT=wt_all[:, col : col + 1],
                        rhs=ht_g[i][:, nb, :],
                        start=(q == 0 and b == 0),
                        stop=(q == NCHUNK - 1 and b == NBLK - 1),
                        skip_group_check=True,
                        tile_position=(0, 32 * i),
                    )
            if q == NCHUNK - 1:
                hts.pop(g)

        prefetch_x3(0)
        for u in range(NU + CTX_LAG):
            if 1 <= u < NU + 1:
                unit_T(u - 1)
            if 2 <= u < NU + 2:
                unit_eT(u - 2)
            if u >= CTX_LAG and u - CTX_LAG < NU:
                unit_ctx(u - CTX_LAG)
            if u < NU:
                unit_conv(u)

        ctx_sb = smallp.tile([128, 512], f32, tag="ctxsb", name="ctxsb")
        for i in range(4):
            nc.scalar.activation(
                ctx_sb[32 * i : 32 * i + 1, :], ctxp[32 * i : 32 * i + 1, :], AF.Copy
            )
        nc.sync.dma_start(out=ctxb_d.ap(), in_=strided4(ctx_sb))

    nc.compile()
    _CACHE["prog"] = nc
    return nc


def _host_consts(emb, conv_w, conv_b, W_a, v_a):
    import ml_dtypes

    # permutation: d' = s*32 + c  <->  d = c*4 + s
    perm = np.array([(dp % 32) * 4 + dp // 32 for dp in range(128)], dtype=np.int64)
    # block-diagonal conv weights: rows (j, 3i+k) -> cols (j, c)
    w15 = conv_w.astype(np.float32).transpose(1, 2, 0).reshape(15, 32)
    W3 = np.zeros((64, 128), dtype=np.float32)
    for j in range(S):
        W3[16 * j : 16 * j + 15, 32 * j : 32 * j + 32] = w15
        W3[16 * j + 15, 32 * j : 32 * j + 32] = conv_b.astype(np.float32)
    gt3 = np.vstack([W3, W3]).astype(ml_dtypes.bfloat16)  # [128, 128]
    W_p = W_a[np.ix_(perm, perm)].astype(np.float32)
    wat = np.ascontiguousarray(W_p.T).astype(ml_dtypes.bfloat16)
    vone = np.ascontiguousarray(
        v_a[perm].astype(np.float32)[:, None]
    ).astype(ml_dtypes.bfloat16)
    return perm, gt3, wat, vone


def _in_maps(input_seq, emb, conv_w, conv_b, W_a, v_a):
    import ml_dtypes

    perm, gt3, wat, vone = _host_consts(emb, conv_w, conv_b, W_a, v_a)
    seq = np.asarray(input_seq).astype(np.int64)
    embf = np.asarray(emb).astype(np.float32)
    x = embf[seq]  # [B, L, 5]
    x3 = np.empty((B, KR, PL), dtype=ml_dtypes.bfloat16)
    for i in range(5):
        for k in range(3):
            x3[:, 3 * i + k, :] = x[:, k : k + PL, i].astype(ml_dtypes.bfloat16)
    x3[:, 15, :] = np.float32(1.0)
    maps = []
    for c in range(NCORES):
        maps.append(
            {
                "x3": np.ascontiguousarray(x3[BPC * c : BPC * (c + 1)]),
                "gt3": gt3,
                "wat": wat,
                "vone": vone,
            }
        )
    return perm, maps


def _assemble_one(perm, res):
    """ctx for one core: ctxb [4, 512] = [i, 128*g + d'], wdump [128, 16*NU]."""
    buf = np.asarray(res["ctxb"], dtype=np.float32)
    ctxb = buf.reshape(4, 4, 128).transpose(1, 0, 2).reshape(BPC, D)
    wd = np.asarray(res["wdump"], dtype=np.float32)
    # col 16*(4g+q) + 4i + b -> batch 4g+i
    wr = wd.reshape(128, NG, NCHUNK, 4, NBLK)  # [n', g, q, i, b]
    zsum = wr.sum(axis=(0, 2, 4)).reshape(BPC)  # [g, i] row-major -> batch 4g+i
    blk = np.empty((BPC, D), dtype=np.float32)
    blk[:, perm] = ctxb / zsum[:, None]
    return blk


def _assemble(perm, results):
    out = np.empty((B, D), dtype=np.float32)
    for c, res in enumerate(results):
        out[BPC * c : BPC * (c + 1)] = _assemble_one(perm, res)
    return out


def kernel(input_seq, emb, conv_w, conv_b, W_a, v_a):
    from concourse import bass_utils

    nc = _build_program()
    perm, maps = _in_maps(input_seq, emb, conv_w, conv_b, W_a, v_a)
    res = bass_utils.run_bass_kernel_spmd(nc, maps, core_ids=list(range(NCORES)))
    return _assemble(perm, res.results)
